# revision 25
# baseline (speedup 1.0000x reference)
"""DeepSeek-style MoE (64 experts, top-8, group-limited routing) on 8 TRN2 cores.

Strategy:
  - Router + dispatch/combine run on host in numpy (exact replica of the
    reference semantics, including capacity drops).
  - Expert-parallel: core c computes 8 routed experts (rank-balanced
    assignment) plus a 512-token shard of the shared expert (as a 9th
    "slot" with identical compute structure).
  - All activations flow in transposed [H, tokens] layout so every GEMM
    contracts over the partition dim with weights used in native layout
    (no on-device transposes).
  - One SPMD program for all 8 cores: slot token-counts are fixed in the
    program (padded); which expert fills a slot is per-core data.
"""

import threading

import numpy as np

import concourse.bass as bass
import concourse.mybir as mybir
import concourse.tile as tile
from concourse import bacc
from concourse.bass_utils import run_bass_kernel_spmd

# ---- problem constants (hardcoded; must match the grader's reference) ----
E, H, I_DIM = 64, 2048, 704
G, TOPK_GROUP, K = 8, 4, 8
B, S = 2, 2048
N = B * S
CAP = 2 * N * K // E
SCALE = 2.5
NCORES = 8
R_SLOTS = E // NCORES       # routed expert slots per core
SLOTS = R_SLOTS + 1         # + shared-expert slot
SH_T = N // NCORES          # shared-expert tokens per core
HCH = H // 128              # 16 h-chunks
I_SIZES = [128] * (I_DIM // 128) + ([I_DIM % 128] if I_DIM % 128 else [])
I_OFFS = np.cumsum([0] + I_SIZES[:-1]).tolist()
NI = len(I_SIZES)

KDT = "bf16"                # "f32r" | "bf16"  (matmul dtype on device)


# ---------------------------------------------------------------- routing --
def _route(x, router_weight, e_bias):
    logits = x.astype(np.float32) @ router_weight.astype(np.float32).T
    scores = 1.0 / (1.0 + np.exp(-logits))
    sc = scores + e_bias[None, :].astype(np.float32)
    n = x.shape[0]
    g = sc.reshape(n, G, E // G)
    top2 = -np.sort(-g, axis=-1)[:, :, :2]
    group_scores = top2.sum(-1)
    grp_idx = np.argsort(-group_scores, axis=-1, kind="stable")[:, :TOPK_GROUP]
    group_mask = np.zeros((n, G), np.float32)
    np.put_along_axis(group_mask, grp_idx, 1.0, axis=1)
    masked = np.where(np.repeat(group_mask, E // G, axis=1) > 0, sc, 0.0)
    topk_idx = np.argsort(-masked, axis=-1, kind="stable")[:, :K].astype(np.int32)
    topk_w = np.take_along_axis(scores, topk_idx, axis=1)
    topk_w = topk_w / (topk_w.sum(-1, keepdims=True) + 1e-20)
    return topk_idx, (topk_w * SCALE).astype(np.float32)


def _dispatch(flat_e):
    """pos[j] = #earlier occurrences of flat_e[j]; matches reference cumsum."""
    nk = flat_e.shape[0]
    order = np.argsort(flat_e, kind="stable")
    sorted_e = flat_e[order]
    counts = np.bincount(flat_e, minlength=E)
    group_start = np.zeros(nk, np.int64)
    starts = np.cumsum(np.concatenate([[0], counts[:-1]]))
    group_start = np.repeat(starts, counts)
    pos_sorted = np.arange(nk) - group_start
    pos = np.empty(nk, np.int64)
    pos[order] = pos_sorted
    valid = pos < CAP
    return pos, valid, counts


# ---------------------------------------------------------- device kernel --
_BUILD_CACHE: dict = {}
_BUILD_LOCK = threading.Lock()


def _np_in_dt():
    if KDT == "bf16":
        import ml_dtypes
        return np.dtype(ml_dtypes.bfloat16)
    return np.dtype(np.float32)


def _np_out_dt():
    return _np_in_dt()


def _pieces(t):
    """Split t columns into <=512 balanced pieces (multiples of 16)."""
    n = -(-t // 512)
    base = -(-t // n)
    base = -(-base // 16) * 16
    out = []
    o = 0
    while o < t:
        p = min(base, t - o)
        out.append((o, p))
        o += p
    return out


def _build(tsizes, reps=1, loop_reps=0):
    """Build + schedule the SPMD program for the given per-slot token counts.

    reps: static unroll count of the whole body (normally 1).
    loop_reps: if >0, wrap the body in a hardware For_i loop with this trip
        count (used only for timing measurements)."""
    key = (tuple(tsizes), KDT, reps, loop_reps)
    with _BUILD_LOCK:
        if key in _BUILD_CACHE:
            return _BUILD_CACHE[key]

    if KDT == "bf16":
        dt_in = mybir.dt.bfloat16
        dt_out = mybir.dt.bfloat16
    else:
        dt_in = mybir.dt.float32r
        dt_out = mybir.dt.float32
    f32 = mybir.dt.float32
    TC = int(sum(tsizes))
    offs = np.cumsum([0] + list(tsizes[:-1])).tolist()
    nchs = [-(-int(t) // 128) for t in tsizes]      # 128-token chunks / slot
    choffs = np.cumsum([0] + nchs[:-1]).tolist()
    NCHT = int(sum(nchs))

    nc = bacc.Bacc(None, target_bir_lowering=False)
    # All DRAM tensors are host-pre-swizzled into the exact SBUF image the
    # kernel wants: partition-major [128, ...] with per-partition-contiguous
    # payloads, so every DMA is a plain 2D copy with large descriptors.
    bufT = nc.dram_tensor("bufT", [128, HCH * TC], dt_in,
                          kind="ExternalInput")
    # per h-block columns: [gate 0:640 | gate 640:704 | up 640:704 | up 0:640]
    wgu = nc.dram_tensor("wgu", [SLOTS, 128, HCH * 2 * I_DIM], dt_in,
                         kind="ExternalInput")
    wd = nc.dram_tensor("wd", [SLOTS, 2, 128, (I_DIM // 128 + 1) * H // 2],
                        dt_in, kind="ExternalInput")
    # token-major output: chunk (s, c) holds rows = tokens c*128+p of slot s,
    # cols = full H
    yT = nc.dram_tensor("yT", [128, NCHT * H], dt_out, kind="ExternalOutput")

    import contextlib

    HHALF = HCH // 2        # GEMM2 wd staged in two ho-halves of 8
    NI5 = NI - 1            # full 128-row i-chunks
    ITAIL = I_DIM - NI5 * 128   # 64: tail rows packed gate|up
    W2 = 2 * I_DIM          # 1408 packed weight columns per h
    GRP = 4                 # output staged/DMAd in groups of 4 ho rows
    with tile.TileContext(nc) as tc:
        with tc.tile_pool(name="bufp", bufs=2) as bufp, \
             tc.tile_pool(name="wgup", bufs=2) as wgup, \
             tc.tile_pool(name="wdp", bufs=2) as wdp, \
             tc.tile_pool(name="htp", bufs=2 * NI) as htp, \
             tc.tile_pool(name="actp", bufs=3) as actp, \
             tc.tile_pool(name="outp", bufs=4) as outp, \
             tc.tile_pool(name="pgp", bufs=2, space="PSUM") as pgp, \
             tc.tile_pool(name="pup", bufs=2, space="PSUM") as pup, \
             tc.tile_pool(name="pyp", bufs=3, space="PSUM") as pyp, \
             (tc.For_i(0, loop_reps, 1) if loop_reps > 0
              else contextlib.nullcontext()):
            for _rep in range(reps):
                for s in range(SLOTS):
                    T = int(tsizes[s])
                    off = offs[s]
                    pieces = _pieces(T)
                    # ---- stage activations + weights on SP; transfers are
                    # split into ~4us chunks so the (modeled-exclusive) DMA
                    # engines interleave them with out/wd traffic ----
                    bt = bufp.tile([128, HCH * T], dt_in, tag="buf",
                                   name=f"bt{s}")
                    for q in range(2):
                        c0 = q * (HCH // 2) * T
                        nc.sync.dma_start(
                            bt[:, c0:c0 + (HCH // 2) * T],
                            bufT[:, HCH * off + c0:
                                 HCH * off + c0 + (HCH // 2) * T])
                    wgt = wgup.tile([128, HCH * W2], dt_in, tag="wgu",
                                    name="wgt")
                    for q in range(4):
                        c0 = q * (HCH // 4) * W2
                        nc.sync.dma_start(
                            wgt[:, c0:c0 + (HCH // 4) * W2],
                            wgu[s, :, c0:c0 + (HCH // 4) * W2])
                    hts = [htp.tile([128, T], dt_in, tag="ht",
                                    name=f"ht{s}_{i}") for i in range(NI)]
                    # ---- GEMM1 (gate & up) + silu*mul, piece-major ----
                    # wgt columns per h: [g 0:640 | g-tail 64 | u-tail 64
                    # | u 0:640]; tail pair handled as one packed 128-row
                    # stationary whose PSUM rows 0:64 are gate, 64:128 up.
                    for p, (t0, tp) in enumerate(pieces):
                        for it in range(NI5):
                            io = it * 128
                            pg = pgp.tile([128, 512], f32, tag="pg",
                                          name="pg")[:, :tp]
                            pu = pup.tile([128, 512], f32, tag="pu",
                                          name="pu")[:, :tp]
                            for h in range(HCH):
                                nc.tensor.matmul(
                                    pg, wgt[:, h * W2 + io:
                                            h * W2 + io + 128],
                                    bt[:, h * T + t0:h * T + t0 + tp],
                                    start=(h == 0), stop=(h == HCH - 1))
                            for h in range(HCH):
                                nc.tensor.matmul(
                                    pu, wgt[:, h * W2 + I_DIM + ITAIL + io:
                                            h * W2 + I_DIM + ITAIL + io + 128],
                                    bt[:, h * T + t0:h * T + t0 + tp],
                                    start=(h == 0), stop=(h == HCH - 1))
                            sil = actp.tile([128, 512], f32, tag="act",
                                            name="sil")[:, :tp]
                            nc.scalar.activation(
                                sil, pg, mybir.ActivationFunctionType.Silu)
                            nc.vector.tensor_mul(
                                hts[it][:, t0:t0 + tp], sil, pu)
                        # packed gate|up tail chunk
                        pt = pgp.tile([128, 512], f32, tag="pg",
                                      name="pt")[:, :tp]
                        for h in range(HCH):
                            nc.tensor.matmul(
                                pt, wgt[:, h * W2 + NI5 * 128:
                                        h * W2 + NI5 * 128 + 2 * ITAIL],
                                bt[:, h * T + t0:h * T + t0 + tp],
                                start=(h == 0), stop=(h == HCH - 1))
                        silt = actp.tile([128, 512], f32, tag="act",
                                         name="silt")[:ITAIL, :tp]
                        nc.scalar.activation(
                            silt, pt[:ITAIL, :tp],
                            mybir.ActivationFunctionType.Silu)
                        nc.vector.tensor_mul(
                            hts[NI5][:ITAIL, t0:t0 + tp], silt,
                            pt[ITAIL:2 * ITAIL, :tp])
                    # wd staged here (not at slot start): its buffers free
                    # late (mid-GEMM2 of the previous slot), and emitting the
                    # loads after GEMM1 keeps that late dependency from
                    # head-of-line-blocking the next slot's prefetch on SP.
                    wds = []
                    for half in range(2):
                        wdh = wdp.tile([128, NI * HHALF * 128], dt_in,
                                       tag="wd", name="wdh")
                        nc.sync.dma_start(wdh[:], wd[s, half])
                        wds.append(wdh)
                    # ---- GEMM2 (down), transposed: token-chunks become the
                    # output partition dim (ht chunk is the stationary, wd
                    # streams) so every matmul is a full 512-wide stream —
                    # fewer, wider matmuls amortize LDWEIGHTS best. Output
                    # comes out token-major; out DMAs on ACT queue ----
                    for c in range(-(-T // 128)):
                        pcz = min(128, T - c * 128)
                        ot = outp.tile([128, H], dt_out, tag="out", name="yo")
                        for half in range(2):
                            for j in range(HHALF * 128 // 512):
                                py = pyp.tile([128, 512], f32, tag="py",
                                              name="py")[:pcz, :]
                                for it in range(NI):
                                    isz = I_SIZES[it]
                                    nc.tensor.matmul(
                                        py,
                                        hts[it][:isz, c * 128:c * 128 + pcz],
                                        wds[half][:isz,
                                                  it * HHALF * 128 + j * 512:
                                                  it * HHALF * 128
                                                  + (j + 1) * 512],
                                        start=(it == 0), stop=(it == NI - 1))
                                hh = (half * 2 + j) * 512
                                nc.vector.tensor_copy(
                                    ot[:pcz, hh:hh + 512], py)
                        nc.scalar.dma_start(
                            yT[:pcz, (choffs[s] + c) * H:
                               (choffs[s] + c + 1) * H],
                            ot[:pcz, :])
    nc.compile()
    with _BUILD_LOCK:
        _BUILD_CACHE[key] = nc
    return nc


# ------------------------------------------------------- jit exec caching --
_EXEC_CACHE: dict = {}


def _get_runner(nc, donate=True):
    """Build (once) a jitted SPMD callable for this nc, mirroring
    bass2jax.run_bass_via_pjrt but reusable across calls."""
    key = (id(nc), donate)
    if key in _EXEC_CACHE:
        return _EXEC_CACHE[key]
    import jax
    from jax.sharding import Mesh, PartitionSpec
    from jax.experimental.shard_map import shard_map
    from concourse import bass2jax

    bass2jax.install_neuronx_cc_hook()

    partition_name = (
        nc.partition_id_tensor.name if nc.partition_id_tensor else None)
    in_names, out_names, out_avals, zero_shapes = [], [], [], []
    for alloc in nc.m.functions[0].allocations:
        if not isinstance(alloc, mybir.MemoryLocationSet):
            continue
        name = alloc.memorylocations[0].name
        if alloc.kind == "ExternalInput":
            if name != partition_name:
                in_names.append(name)
        elif alloc.kind == "ExternalOutput":
            shape = tuple(alloc.tensor_shape)
            dtype = mybir.dt.np(alloc.dtype)
            out_names.append(name)
            out_avals.append(jax.core.ShapedArray(shape, dtype))
            zero_shapes.append((shape, dtype))
    n_params = len(in_names)
    all_names = list(in_names) + list(out_names)
    if partition_name is not None:
        all_names.append(partition_name)

    def _body(*args):
        operands = list(args)
        if partition_name is not None:
            operands.append(bass2jax.partition_id_tensor())
        outs = bass2jax._bass_exec_p.bind(
            *operands,
            out_avals=tuple(out_avals),
            in_names=tuple(all_names),
            out_names=tuple(out_names),
            lowering_input_output_aliases=(),
            sim_require_finite=True,
            sim_require_nnan=True,
            nc=nc,
        )
        return tuple(outs)

    devices = jax.devices()[:NCORES]
    mesh = Mesh(np.asarray(devices), ("core",))
    n_outs = len(out_names)
    sharded = jax.jit(
        shard_map(
            _body, mesh=mesh,
            in_specs=(PartitionSpec("core"),) * (n_params + n_outs),
            out_specs=(PartitionSpec("core"),) * n_outs,
            check_rep=False,
        ),
        donate_argnums=(tuple(range(n_params, n_params + n_outs))
                        if donate else ()),
        keep_unused=True,
    )

    def run(in_maps):
        concat_in = [
            np.concatenate([np.asarray(m[name]) for m in in_maps], axis=0)
            for name in in_names
        ]
        concat_zeros = [
            np.zeros((NCORES * sh[0], *sh[1:]), dt) for sh, dt in zero_shapes
        ]
        out_arrs = sharded(*concat_in, *concat_zeros)
        return [
            {name: np.asarray(out_arrs[i]).reshape(NCORES, *out_avals[i].shape)[c]
             for i, name in enumerate(out_names)}
            for c in range(NCORES)
        ]

    def put(in_maps):
        """device_put all inputs (+ zero out-buffers) once; returns args list."""
        from jax.sharding import NamedSharding
        concat_in = [
            np.concatenate([np.asarray(m[name]) for m in in_maps], axis=0)
            for name in in_names
        ]
        concat_zeros = [
            np.zeros((NCORES * sh[0], *sh[1:]), dt) for sh, dt in zero_shapes
        ]
        sh = NamedSharding(mesh, PartitionSpec("core"))
        return [jax.device_put(a, sh) for a in concat_in + concat_zeros]

    def run_resident(args):
        """Execute on device-resident args; returns jax arrays (no download)."""
        out = sharded(*args)
        jax.block_until_ready(out)
        return out

    run.put = put
    run.run_resident = run_resident
    _EXEC_CACHE[key] = run
    return run


# ------------------------------------------------------------- host glue --
def _plan(counts):
    """Assign experts to (core, slot) rank-balanced; compute padded sizes.

    Returns experts[c][s] -> expert id, tsizes[SLOTS] (shared last)."""
    counts_eff = np.minimum(counts, CAP)
    order = np.argsort(-counts_eff, kind="stable")
    experts = [[0] * R_SLOTS for _ in range(NCORES)]
    tsizes = []
    for s in range(R_SLOTS):
        grp = order[s * NCORES:(s + 1) * NCORES]
        for c in range(NCORES):
            experts[c][s] = int(grp[c])
        t = int(np.max(counts_eff[grp]))
        t = max(32, -(-t // 16) * 16)
        tsizes.append(t)
    tsizes.append(SH_T)
    return experts, tsizes


def _prepare_inputs(x, inputs, experts, tsizes, pos, valid, flat_e):
    """Build per-core bufT/weight arrays."""
    in_dt = _np_in_dt()
    TC = int(sum(tsizes))
    offs = np.cumsum([0] + list(tsizes[:-1])).astype(np.int64)

    # expert -> (core, slot)
    e2cs = np.zeros((E, 2), np.int64)
    for c in range(NCORES):
        for s in range(R_SLOTS):
            e2cs[experts[c][s]] = (c, s)

    tokens = np.repeat(np.arange(N), K)
    v_idx = np.nonzero(valid)[0]
    ve = flat_e[v_idx]
    vcore = e2cs[ve, 0]
    vslot = e2cs[ve, 1]
    vcol = offs[vslot] + pos[v_idx]

    wg_f, wu_f, wd_f = inputs["w_gate"], inputs["w_up"], inputs["w_down"]
    sh_g, sh_u, sh_d = inputs["sh_gate"], inputs["sh_up"], inputs["sh_down"]
    xT = np.ascontiguousarray(x.T)

    def swiz(a, rows=128):
        """[n*rows, m] -> SBUF image [rows, n*m] (partition-major blocks)."""
        n = a.shape[0] // rows
        return np.ascontiguousarray(
            a.reshape(n, rows, a.shape[1]).swapaxes(0, 1).reshape(
                rows, n * a.shape[1]))

    itail = I_DIM - (I_DIM // 128) * 128 if I_DIM % 128 else 128
    nfull = I_DIM - itail
    HW2 = H // 2

    in_maps = []
    for c in range(NCORES):
        buf = np.zeros((H, TC), in_dt)
        mask = vcore == c
        cols = vcol[mask]
        toks = tokens[v_idx[mask]]
        buf[:, cols] = xT[:, toks]
        buf[:, offs[R_SLOTS]:offs[R_SLOTS] + SH_T] = \
            xT[:, c * SH_T:(c + 1) * SH_T]
        # device bufT: per slot the exact SBUF image [128, HCH*T]
        bufd = np.empty((128, HCH * TC), in_dt)
        for s in range(SLOTS):
            off, t = int(offs[s]), int(tsizes[s])
            bufd[:, HCH * off:HCH * (off + t)] = swiz(buf[:, off:off + t])
        el = experts[c]
        # packed gate/up layout per h-row: [g 0:640 | g 640:704 | u 640:704
        # | u 0:640] — lets the two 64-wide tails share one PE stationary.
        wgc = np.empty((SLOTS, 128, HCH * 2 * I_DIM), in_dt)
        wdc = np.zeros((SLOTS, 2, 128, (I_DIM // 128 + 1) * HW2), in_dt)
        pack = np.empty((H, 2 * I_DIM), in_dt)
        for s in range(SLOTS):
            ge = wg_f[el[s]] if s < R_SLOTS else sh_g
            ue = wu_f[el[s]] if s < R_SLOTS else sh_u
            pack[:, :nfull] = ge[:, :nfull]
            pack[:, nfull:nfull + itail] = ge[:, nfull:]
            pack[:, I_DIM:I_DIM + itail] = ue[:, nfull:]
            pack[:, I_DIM + itail:] = ue[:, :nfull]
            wgc[s] = swiz(pack)
            wde = wd_f[el[s]] if s < R_SLOTS else sh_d
            for half in range(2):
                wh = wde[:, half * HW2:(half + 1) * HW2]
                wdc[s, half, :, :nfull * HW2 // 128] = swiz(wh[:nfull])
                wdc[s, half, :itail, nfull * HW2 // 128:] = wh[nfull:]
        in_maps.append({"bufT": bufd, "wgu": wgc, "wd": wdc})
    return in_maps, offs, (vcore, vcol, v_idx)


def _combine(results, offs, tsizes, gather, topk_w, valid):
    TC = int(sum(tsizes))
    nchs = [-(-int(t) // 128) for t in tsizes]
    choffs = np.cumsum([0] + nchs[:-1]).astype(np.int64)
    ys = []
    for c in range(NCORES):
        yd = np.asarray(results[c]["yT"])          # [128, NCHT*H] token-major
        y = np.empty((TC, H), np.float32)
        for s in range(SLOTS):
            t = int(tsizes[s])
            o0 = int(offs[s])
            for ch in range(nchs[s]):
                pcz = min(128, t - ch * 128)
                blk = yd[:pcz, (choffs[s] + ch) * H:(choffs[s] + ch + 1) * H]
                y[o0 + ch * 128:o0 + ch * 128 + pcz] = blk
        ys.append(y)
    # token-major view: [8*TC, H]
    Yt = np.concatenate(ys, axis=0)

    vcore, vcol, v_idx = gather
    w_flat = (topk_w.reshape(-1) * valid.astype(np.float32))
    gcol = np.zeros(N * K, np.int64)
    gcol[v_idx] = vcore * TC + vcol
    routed = Yt[gcol] * w_flat[:, None]
    out = routed.reshape(N, K, H).sum(1)
    # shared expert rows
    sh0 = offs[R_SLOTS]
    for c in range(NCORES):
        out[c * SH_T:(c + 1) * SH_T] += Yt[c * TC + sh0:c * TC + sh0 + SH_T]
    return out


def kernel(**inputs):
    x = np.asarray(inputs["hidden_states"], np.float32).reshape(N, H)
    topk_idx, topk_w = _route(
        x, np.asarray(inputs["router_weight"]), np.asarray(inputs["e_bias"]))
    flat_e = topk_idx.reshape(-1).astype(np.int64)
    pos, valid, counts = _dispatch(flat_e)
    experts, tsizes = _plan(counts)

    np_inputs = {k: np.asarray(v) for k, v in inputs.items()}
    in_maps, offs, gather = _prepare_inputs(
        x, np_inputs, experts, tsizes, pos, valid, flat_e)

    nc = _build(tsizes, reps=1)
    run = _get_runner(nc)
    results = run(in_maps)

    out = _combine(results, offs, tsizes, gather, topk_w, valid)
    return out.reshape(B, S, H).astype(np.float32)


# Expose internals for test.py
run_spmd_raw = run_bass_kernel_spmd



# revision 28
# speedup vs baseline: 1.0239x; 1.0239x over previous
"""DeepSeek-style MoE (64 experts, top-8, group-limited routing) on 8 TRN2 cores.

Strategy:
  - Router + dispatch/combine run on host in numpy (exact replica of the
    reference semantics, including capacity drops).
  - Expert-parallel: core c computes 8 routed experts (rank-balanced
    assignment) plus a 512-token shard of the shared expert (as a 9th
    "slot" with identical compute structure).
  - All activations flow in transposed [H, tokens] layout so every GEMM
    contracts over the partition dim with weights used in native layout
    (no on-device transposes).
  - One SPMD program for all 8 cores: slot token-counts are fixed in the
    program (padded); which expert fills a slot is per-core data.
"""

import threading

import numpy as np

import concourse.bass as bass
import concourse.mybir as mybir
import concourse.tile as tile
from concourse import bacc
from concourse.bass_utils import run_bass_kernel_spmd

# ---- problem constants (hardcoded; must match the grader's reference) ----
E, H, I_DIM = 64, 2048, 704
G, TOPK_GROUP, K = 8, 4, 8
B, S = 2, 2048
N = B * S
CAP = 2 * N * K // E
SCALE = 2.5
NCORES = 8
R_SLOTS = E // NCORES       # routed expert slots per core
SLOTS = R_SLOTS + 1         # + shared-expert slot
SH_T = N // NCORES          # shared-expert tokens per core
HCH = H // 128              # 16 h-chunks
I_SIZES = [128] * (I_DIM // 128) + ([I_DIM % 128] if I_DIM % 128 else [])
I_OFFS = np.cumsum([0] + I_SIZES[:-1]).tolist()
NI = len(I_SIZES)

KDT = "bf16"                # "f32r" | "bf16"  (matmul dtype on device)


# ---------------------------------------------------------------- routing --
def _route(x, router_weight, e_bias):
    logits = x.astype(np.float32) @ router_weight.astype(np.float32).T
    scores = 1.0 / (1.0 + np.exp(-logits))
    sc = scores + e_bias[None, :].astype(np.float32)
    n = x.shape[0]
    g = sc.reshape(n, G, E // G)
    top2 = -np.sort(-g, axis=-1)[:, :, :2]
    group_scores = top2.sum(-1)
    grp_idx = np.argsort(-group_scores, axis=-1, kind="stable")[:, :TOPK_GROUP]
    group_mask = np.zeros((n, G), np.float32)
    np.put_along_axis(group_mask, grp_idx, 1.0, axis=1)
    masked = np.where(np.repeat(group_mask, E // G, axis=1) > 0, sc, 0.0)
    topk_idx = np.argsort(-masked, axis=-1, kind="stable")[:, :K].astype(np.int32)
    topk_w = np.take_along_axis(scores, topk_idx, axis=1)
    topk_w = topk_w / (topk_w.sum(-1, keepdims=True) + 1e-20)
    return topk_idx, (topk_w * SCALE).astype(np.float32)


def _dispatch(flat_e):
    """pos[j] = #earlier occurrences of flat_e[j]; matches reference cumsum."""
    nk = flat_e.shape[0]
    order = np.argsort(flat_e, kind="stable")
    sorted_e = flat_e[order]
    counts = np.bincount(flat_e, minlength=E)
    group_start = np.zeros(nk, np.int64)
    starts = np.cumsum(np.concatenate([[0], counts[:-1]]))
    group_start = np.repeat(starts, counts)
    pos_sorted = np.arange(nk) - group_start
    pos = np.empty(nk, np.int64)
    pos[order] = pos_sorted
    valid = pos < CAP
    return pos, valid, counts


# ---------------------------------------------------------- device kernel --
_BUILD_CACHE: dict = {}
_BUILD_LOCK = threading.Lock()


def _np_in_dt():
    if KDT == "bf16":
        import ml_dtypes
        return np.dtype(ml_dtypes.bfloat16)
    return np.dtype(np.float32)


def _np_out_dt():
    return _np_in_dt()


def _pieces(t):
    """Split t columns into <=512 balanced pieces (multiples of 16)."""
    n = -(-t // 512)
    base = -(-t // n)
    base = -(-base // 16) * 16
    out = []
    o = 0
    while o < t:
        p = min(base, t - o)
        out.append((o, p))
        o += p
    return out


def _build(tsizes, reps=1, loop_reps=0):
    """Build + schedule the SPMD program for the given per-slot token counts.

    reps: static unroll count of the whole body (normally 1).
    loop_reps: if >0, wrap the body in a hardware For_i loop with this trip
        count (used only for timing measurements)."""
    key = (tuple(tsizes), KDT, reps, loop_reps)
    with _BUILD_LOCK:
        if key in _BUILD_CACHE:
            return _BUILD_CACHE[key]

    if KDT == "bf16":
        dt_in = mybir.dt.bfloat16
        dt_out = mybir.dt.bfloat16
    else:
        dt_in = mybir.dt.float32r
        dt_out = mybir.dt.float32
    f32 = mybir.dt.float32
    TC = int(sum(tsizes))
    offs = np.cumsum([0] + list(tsizes[:-1])).tolist()

    nc = bacc.Bacc(None, target_bir_lowering=False)
    # All DRAM tensors are host-pre-swizzled into the exact SBUF image the
    # kernel wants: partition-major [128, ...] with per-partition-contiguous
    # payloads, so every DMA is a plain 2D copy with large descriptors.
    bufT = nc.dram_tensor("bufT", [128, HCH * TC], dt_in,
                          kind="ExternalInput")
    # per h-block columns: [gate 0:640 | gate 640:704 | up 640:704 | up 0:640]
    wgu = nc.dram_tensor("wgu", [SLOTS, 128, HCH * 2 * I_DIM], dt_in,
                         kind="ExternalInput")
    wd = nc.dram_tensor("wd", [SLOTS, 2, 128, (I_DIM // 128 + 1) * H // 2],
                        dt_in, kind="ExternalInput")
    yT = nc.dram_tensor("yT", [128, HCH * TC], dt_out, kind="ExternalOutput")

    import contextlib

    HHALF = HCH // 2        # GEMM2 wd staged in two ho-halves of 8
    NI5 = NI - 1            # full 128-row i-chunks
    ITAIL = I_DIM - NI5 * 128   # 64: tail rows packed gate|up
    W2 = 2 * I_DIM          # 1408 packed weight columns per h
    GRP = 4                 # output staged/DMAd in groups of 4 ho rows
    with tile.TileContext(nc) as tc:
        with tc.tile_pool(name="bufp", bufs=2) as bufp, \
             tc.tile_pool(name="wgup", bufs=2) as wgup, \
             tc.tile_pool(name="wdp", bufs=2) as wdp, \
             tc.tile_pool(name="htp", bufs=2 * NI) as htp, \
             tc.tile_pool(name="actp", bufs=3) as actp, \
             tc.tile_pool(name="outp", bufs=4) as outp, \
             tc.tile_pool(name="pgp", bufs=2, space="PSUM") as pgp, \
             tc.tile_pool(name="pup", bufs=2, space="PSUM") as pup, \
             tc.tile_pool(name="pyp", bufs=3, space="PSUM") as pyp, \
             (tc.For_i(0, loop_reps, 1) if loop_reps > 0
              else contextlib.nullcontext()):
            for _rep in range(reps):
                for s in range(SLOTS):
                    T = int(tsizes[s])
                    off = offs[s]
                    pieces = _pieces(T)
                    # ---- stage activations + weights on SP; transfers are
                    # split into ~4us chunks so the (modeled-exclusive) DMA
                    # engines interleave them with out/wd traffic ----
                    bt = bufp.tile([128, HCH * T], dt_in, tag="buf",
                                   name=f"bt{s}")
                    for q in range(2):
                        c0 = q * (HCH // 2) * T
                        nc.sync.dma_start(
                            bt[:, c0:c0 + (HCH // 2) * T],
                            bufT[:, HCH * off + c0:
                                 HCH * off + c0 + (HCH // 2) * T])
                    wgt = wgup.tile([128, HCH * W2], dt_in, tag="wgu",
                                    name="wgt")
                    for q in range(4):
                        c0 = q * (HCH // 4) * W2
                        nc.sync.dma_start(
                            wgt[:, c0:c0 + (HCH // 4) * W2],
                            wgu[s, :, c0:c0 + (HCH // 4) * W2])
                    hts = [htp.tile([128, T], dt_in, tag="ht",
                                    name=f"ht{s}_{i}") for i in range(NI)]
                    # ---- GEMM1 (gate & up) + silu*mul, piece-major ----
                    # wgt columns per h: [g 0:640 | g-tail 64 | u-tail 64
                    # | u 0:640]; tail pair handled as one packed 128-row
                    # stationary whose PSUM rows 0:64 are gate, 64:128 up.
                    for p, (t0, tp) in enumerate(pieces):
                        for it in range(NI5):
                            io = it * 128
                            pg = pgp.tile([128, 512], f32, tag="pg",
                                          name="pg")[:, :tp]
                            pu = pup.tile([128, 512], f32, tag="pu",
                                          name="pu")[:, :tp]
                            for h in range(HCH):
                                nc.tensor.matmul(
                                    pg, wgt[:, h * W2 + io:
                                            h * W2 + io + 128],
                                    bt[:, h * T + t0:h * T + t0 + tp],
                                    start=(h == 0), stop=(h == HCH - 1))
                            for h in range(HCH):
                                nc.tensor.matmul(
                                    pu, wgt[:, h * W2 + I_DIM + ITAIL + io:
                                            h * W2 + I_DIM + ITAIL + io + 128],
                                    bt[:, h * T + t0:h * T + t0 + tp],
                                    start=(h == 0), stop=(h == HCH - 1))
                            sil = actp.tile([128, 512], f32, tag="act",
                                            name="sil")[:, :tp]
                            nc.scalar.activation(
                                sil, pg, mybir.ActivationFunctionType.Silu)
                            nc.vector.tensor_mul(
                                hts[it][:, t0:t0 + tp], sil, pu)
                        # packed gate|up tail chunk
                        pt = pgp.tile([128, 512], f32, tag="pg",
                                      name="pt")[:, :tp]
                        for h in range(HCH):
                            nc.tensor.matmul(
                                pt, wgt[:, h * W2 + NI5 * 128:
                                        h * W2 + NI5 * 128 + 2 * ITAIL],
                                bt[:, h * T + t0:h * T + t0 + tp],
                                start=(h == 0), stop=(h == HCH - 1))
                        silt = actp.tile([128, 512], f32, tag="act",
                                         name="silt")[:ITAIL, :tp]
                        nc.scalar.activation(
                            silt, pt[:ITAIL, :tp],
                            mybir.ActivationFunctionType.Silu)
                        nc.vector.tensor_mul(
                            hts[NI5][:ITAIL, t0:t0 + tp], silt,
                            pt[ITAIL:2 * ITAIL, :tp])
                    # wd staged here (not at slot start): its buffers free
                    # late (mid-GEMM2 of the previous slot), and emitting the
                    # loads after GEMM1 keeps that late dependency from
                    # head-of-line-blocking the next slot's prefetch on SP.
                    wds = []
                    for half in range(2):
                        wdh = wdp.tile([128, NI * HHALF * 128], dt_in,
                                       tag="wd", name="wdh")
                        nc.sync.dma_start(wdh[:], wd[s, half])
                        wds.append(wdh)
                    # ---- GEMM2 (down), ho-major; out DMAs on ACT queue ----
                    yo = None
                    for ho in range(HCH):
                        half = ho // HHALF
                        hcol = (ho % HHALF) * 128
                        if ho % GRP == 0:
                            yo = outp.tile([128, GRP * T], dt_out, tag="out",
                                           name="yo")
                        for p, (t0, tp) in enumerate(pieces):
                            py = pyp.tile([128, 512], f32, tag="py",
                                          name="py")[:, :tp]
                            for it in range(NI):
                                isz = I_SIZES[it]
                                nc.tensor.matmul(
                                    py,
                                    wds[half][:isz, it * HHALF * 128 + hcol:
                                              it * HHALF * 128 + hcol + 128],
                                    hts[it][:isz, t0:t0 + tp],
                                    start=(it == 0), stop=(it == NI - 1))
                            nc.vector.tensor_copy(
                                yo[:, (ho % GRP) * T + t0:
                                   (ho % GRP) * T + t0 + tp], py)
                        if ho % GRP == GRP - 1:
                            g0 = ho - GRP + 1
                            # out DMAs ride the otherwise-idle Pool SWDGE
                            # queue: ~25ns issue cost and no head-of-line
                            # contention with the ACT queue's silu stream
                            nc.gpsimd.dma_start(
                                yT[:, HCH * off + g0 * T:
                                   HCH * off + (g0 + GRP) * T],
                                yo[:])
    nc.compile()
    with _BUILD_LOCK:
        _BUILD_CACHE[key] = nc
    return nc


# ------------------------------------------------------- jit exec caching --
_EXEC_CACHE: dict = {}


def _get_runner(nc, donate=True):
    """Build (once) a jitted SPMD callable for this nc, mirroring
    bass2jax.run_bass_via_pjrt but reusable across calls."""
    key = (id(nc), donate)
    if key in _EXEC_CACHE:
        return _EXEC_CACHE[key]
    import jax
    from jax.sharding import Mesh, PartitionSpec
    from jax.experimental.shard_map import shard_map
    from concourse import bass2jax

    bass2jax.install_neuronx_cc_hook()

    partition_name = (
        nc.partition_id_tensor.name if nc.partition_id_tensor else None)
    in_names, out_names, out_avals, zero_shapes = [], [], [], []
    for alloc in nc.m.functions[0].allocations:
        if not isinstance(alloc, mybir.MemoryLocationSet):
            continue
        name = alloc.memorylocations[0].name
        if alloc.kind == "ExternalInput":
            if name != partition_name:
                in_names.append(name)
        elif alloc.kind == "ExternalOutput":
            shape = tuple(alloc.tensor_shape)
            dtype = mybir.dt.np(alloc.dtype)
            out_names.append(name)
            out_avals.append(jax.core.ShapedArray(shape, dtype))
            zero_shapes.append((shape, dtype))
    n_params = len(in_names)
    all_names = list(in_names) + list(out_names)
    if partition_name is not None:
        all_names.append(partition_name)

    def _body(*args):
        operands = list(args)
        if partition_name is not None:
            operands.append(bass2jax.partition_id_tensor())
        outs = bass2jax._bass_exec_p.bind(
            *operands,
            out_avals=tuple(out_avals),
            in_names=tuple(all_names),
            out_names=tuple(out_names),
            lowering_input_output_aliases=(),
            sim_require_finite=True,
            sim_require_nnan=True,
            nc=nc,
        )
        return tuple(outs)

    devices = jax.devices()[:NCORES]
    mesh = Mesh(np.asarray(devices), ("core",))
    n_outs = len(out_names)
    sharded = jax.jit(
        shard_map(
            _body, mesh=mesh,
            in_specs=(PartitionSpec("core"),) * (n_params + n_outs),
            out_specs=(PartitionSpec("core"),) * n_outs,
            check_rep=False,
        ),
        donate_argnums=(tuple(range(n_params, n_params + n_outs))
                        if donate else ()),
        keep_unused=True,
    )

    def run(in_maps):
        concat_in = [
            np.concatenate([np.asarray(m[name]) for m in in_maps], axis=0)
            for name in in_names
        ]
        concat_zeros = [
            np.zeros((NCORES * sh[0], *sh[1:]), dt) for sh, dt in zero_shapes
        ]
        out_arrs = sharded(*concat_in, *concat_zeros)
        return [
            {name: np.asarray(out_arrs[i]).reshape(NCORES, *out_avals[i].shape)[c]
             for i, name in enumerate(out_names)}
            for c in range(NCORES)
        ]

    def put(in_maps):
        """device_put all inputs (+ zero out-buffers) once; returns args list."""
        from jax.sharding import NamedSharding
        concat_in = [
            np.concatenate([np.asarray(m[name]) for m in in_maps], axis=0)
            for name in in_names
        ]
        concat_zeros = [
            np.zeros((NCORES * sh[0], *sh[1:]), dt) for sh, dt in zero_shapes
        ]
        sh = NamedSharding(mesh, PartitionSpec("core"))
        return [jax.device_put(a, sh) for a in concat_in + concat_zeros]

    def run_resident(args):
        """Execute on device-resident args; returns jax arrays (no download)."""
        out = sharded(*args)
        jax.block_until_ready(out)
        return out

    run.put = put
    run.run_resident = run_resident
    _EXEC_CACHE[key] = run
    return run


# ------------------------------------------------------------- host glue --
def _plan(counts):
    """Assign experts to (core, slot) rank-balanced; compute padded sizes.

    Returns experts[c][s] -> expert id, tsizes[SLOTS] (shared last)."""
    counts_eff = np.minimum(counts, CAP)
    order = np.argsort(-counts_eff, kind="stable")
    experts = [[0] * R_SLOTS for _ in range(NCORES)]
    tsizes = []
    for s in range(R_SLOTS):
        grp = order[s * NCORES:(s + 1) * NCORES]
        for c in range(NCORES):
            experts[c][s] = int(grp[c])
        t = int(np.max(counts_eff[grp]))
        t = max(32, -(-t // 4) * 4)
        tsizes.append(t)
    tsizes.append(SH_T)
    return experts, tsizes


def _prepare_inputs(x, inputs, experts, tsizes, pos, valid, flat_e):
    """Build per-core bufT/weight arrays."""
    in_dt = _np_in_dt()
    TC = int(sum(tsizes))
    offs = np.cumsum([0] + list(tsizes[:-1])).astype(np.int64)

    # expert -> (core, slot)
    e2cs = np.zeros((E, 2), np.int64)
    for c in range(NCORES):
        for s in range(R_SLOTS):
            e2cs[experts[c][s]] = (c, s)

    tokens = np.repeat(np.arange(N), K)
    v_idx = np.nonzero(valid)[0]
    ve = flat_e[v_idx]
    vcore = e2cs[ve, 0]
    vslot = e2cs[ve, 1]
    vcol = offs[vslot] + pos[v_idx]

    wg_f, wu_f, wd_f = inputs["w_gate"], inputs["w_up"], inputs["w_down"]
    sh_g, sh_u, sh_d = inputs["sh_gate"], inputs["sh_up"], inputs["sh_down"]
    xT = np.ascontiguousarray(x.T)

    def swiz(a, rows=128):
        """[n*rows, m] -> SBUF image [rows, n*m] (partition-major blocks)."""
        n = a.shape[0] // rows
        return np.ascontiguousarray(
            a.reshape(n, rows, a.shape[1]).swapaxes(0, 1).reshape(
                rows, n * a.shape[1]))

    itail = I_DIM - (I_DIM // 128) * 128 if I_DIM % 128 else 128
    nfull = I_DIM - itail
    HW2 = H // 2

    in_maps = []
    for c in range(NCORES):
        buf = np.zeros((H, TC), in_dt)
        mask = vcore == c
        cols = vcol[mask]
        toks = tokens[v_idx[mask]]
        buf[:, cols] = xT[:, toks]
        buf[:, offs[R_SLOTS]:offs[R_SLOTS] + SH_T] = \
            xT[:, c * SH_T:(c + 1) * SH_T]
        # device bufT: per slot the exact SBUF image [128, HCH*T]
        bufd = np.empty((128, HCH * TC), in_dt)
        for s in range(SLOTS):
            off, t = int(offs[s]), int(tsizes[s])
            bufd[:, HCH * off:HCH * (off + t)] = swiz(buf[:, off:off + t])
        el = experts[c]
        # packed gate/up layout per h-row: [g 0:640 | g 640:704 | u 640:704
        # | u 0:640] — lets the two 64-wide tails share one PE stationary.
        wgc = np.empty((SLOTS, 128, HCH * 2 * I_DIM), in_dt)
        wdc = np.zeros((SLOTS, 2, 128, (I_DIM // 128 + 1) * HW2), in_dt)
        pack = np.empty((H, 2 * I_DIM), in_dt)
        for s in range(SLOTS):
            ge = wg_f[el[s]] if s < R_SLOTS else sh_g
            ue = wu_f[el[s]] if s < R_SLOTS else sh_u
            pack[:, :nfull] = ge[:, :nfull]
            pack[:, nfull:nfull + itail] = ge[:, nfull:]
            pack[:, I_DIM:I_DIM + itail] = ue[:, nfull:]
            pack[:, I_DIM + itail:] = ue[:, :nfull]
            wgc[s] = swiz(pack)
            wde = wd_f[el[s]] if s < R_SLOTS else sh_d
            for half in range(2):
                wh = wde[:, half * HW2:(half + 1) * HW2]
                wdc[s, half, :, :nfull * HW2 // 128] = swiz(wh[:nfull])
                wdc[s, half, :itail, nfull * HW2 // 128:] = wh[nfull:]
        in_maps.append({"bufT": bufd, "wgu": wgc, "wd": wdc})
    return in_maps, offs, (vcore, vcol, v_idx)


def _combine(results, offs, gather, topk_w, valid):
    TC = None
    ys = []
    for c in range(NCORES):
        yd = np.asarray(results[c]["yT"])          # [128, HCH*TC] swizzled
        TC = yd.shape[1] // HCH
        y = np.empty((H, TC), np.float32)
        bounds = list(offs) + [TC]
        for s in range(SLOTS):
            o0, o1 = int(bounds[s]), int(bounds[s + 1])
            t = o1 - o0
            y[:, o0:o1] = (
                yd[:, HCH * o0:HCH * o1].astype(np.float32, copy=False)
                .reshape(128, HCH, t).swapaxes(0, 1).reshape(H, t))
        ys.append(y)
    # token-major view: [8*TC, H]
    Yt = np.concatenate([y.T for y in ys], axis=0)

    vcore, vcol, v_idx = gather
    w_flat = (topk_w.reshape(-1) * valid.astype(np.float32))
    gcol = np.zeros(N * K, np.int64)
    gcol[v_idx] = vcore * TC + vcol
    routed = Yt[gcol] * w_flat[:, None]
    out = routed.reshape(N, K, H).sum(1)
    # shared expert rows
    sh0 = offs[R_SLOTS]
    for c in range(NCORES):
        out[c * SH_T:(c + 1) * SH_T] += Yt[c * TC + sh0:c * TC + sh0 + SH_T]
    return out


def kernel(**inputs):
    x = np.asarray(inputs["hidden_states"], np.float32).reshape(N, H)
    topk_idx, topk_w = _route(
        x, np.asarray(inputs["router_weight"]), np.asarray(inputs["e_bias"]))
    flat_e = topk_idx.reshape(-1).astype(np.int64)
    pos, valid, counts = _dispatch(flat_e)
    experts, tsizes = _plan(counts)

    np_inputs = {k: np.asarray(v) for k, v in inputs.items()}
    in_maps, offs, gather = _prepare_inputs(
        x, np_inputs, experts, tsizes, pos, valid, flat_e)

    nc = _build(tsizes, reps=1)
    run = _get_runner(nc)
    results = run(in_maps)

    out = _combine(results, offs, gather, topk_w, valid)
    return out.reshape(B, S, H).astype(np.float32)


# Expose internals for test.py
run_spmd_raw = run_bass_kernel_spmd

